# revision 1
# baseline (speedup 1.0000x reference)
"""AdaMoLE (LoRA-MoE routing) Trainium2 kernel, data-parallel over tokens on 8 cores.

Math (per token n):
    logits = x @ Wr.T + br                 [E]
    gate   = softmax(logits)
    thr    = sigmoid(x @ Wt.T + bt) / E    [1]
    w      = relu(gate - thr); w /= max(sum(w), eps)
    h      = x @ A_all                     [E*R]   (A_all = concat_e A_e)
    out    = (h * rep(w) * SCALING) @ B_all        (B_all = concat_e B_e)

Each core takes 2048 tokens. x is passed host-side in a transposed,
chunk-major layout so the contraction dim (d) lands on SBUF partitions and
every DMA is contiguous; f32 -> bf16 cast happens inline in the SWDGE DMA.
Cross-expert (partition-dim) softmax reductions are done with tiny PE
ones-matmuls; weight broadcast across the er=128 lanes with a replication
matmul. Output written f32.
"""

import sys

sys.path.insert(0, "/opt/trn_rl_repo")

import numpy as np
import ml_dtypes

import concourse.bacc as bacc
import concourse.mybir as mybir
import concourse.tile as tile
from concourse.bass_utils import run_bass_kernel_spmd
from contextlib import ExitStack

F32 = mybir.dt.float32
BF16 = mybir.dt.bfloat16
AF = mybir.ActivationFunctionType

B, S, D, DOUT = 4, 4096, 4096, 4096
R, E = 16, 8
SCALING = 8.0 / R  # lora_alpha / r
NCORES = 8
N = B * S
NTOK = N // NCORES        # 2048 tokens per core
# token blocks per core; fine 128-token blocks minimize pipeline fill/drain
# (HW A/B: finer blocking beat coarse 512-token blocks consistently)
BLOCKS = [128] * 16
NBLK = len(BLOCKS)
TBLK = 512                # max block size (psum/sbuf tile sizing)
NDC = D // 128            # 32 contraction chunks
ER = E * R                # 128
NOB = DOUT // 512         # 8 output column blocks

_CACHE = {}


def _build(reps=1, loop=False):
    nc = bacc.Bacc("TRN2", debug=False, num_devices=NCORES)

    X = nc.declare_dram_parameter("X", [128, NDC * NTOK], F32, isOutput=False)
    Aw = nc.declare_dram_parameter("Aw", [128, NDC * ER], BF16, isOutput=False)
    Wc = nc.declare_dram_parameter("Wc", [128, NDC * 9], BF16, isOutput=False)
    Bl = nc.declare_dram_parameter("Bl", [ER, DOUT], BF16, isOutput=False)
    REPs = nc.declare_dram_parameter("REPs", [E, ER], F32, isOutput=False)
    ONES8 = nc.declare_dram_parameter("ONES8", [E, 1], F32, isOutput=False)
    SEL9 = nc.declare_dram_parameter("SEL9", [9, E], F32, isOutput=False)
    BC1 = nc.declare_dram_parameter("BC1", [1, E], F32, isOutput=False)
    BR = nc.declare_dram_parameter("BR", [E, 1], F32, isOutput=False)
    BT8 = nc.declare_dram_parameter("BT8", [E, 1], F32, isOutput=False)
    OUT = nc.declare_dram_parameter("out", [NTOK, DOUT], F32, isOutput=True)

    with tile.TileContext(nc) as tc, ExitStack() as ctx:
        wpool = ctx.enter_context(tc.tile_pool(name="w", bufs=1))
        xpool = ctx.enter_context(tc.tile_pool(name="x", bufs=6))
        opool = ctx.enter_context(tc.tile_pool(name="o", bufs=4))
        spool = ctx.enter_context(tc.tile_pool(name="s", bufs=2))
        hwpool = ctx.enter_context(tc.tile_pool(name="hw", bufs=2))
        ph = ctx.enter_context(tc.tile_pool(name="ph", bufs=2, space="PSUM"))
        pr = ctx.enter_context(tc.tile_pool(name="pr", bufs=1, space="PSUM"))
        pm = ctx.enter_context(tc.tile_pool(name="pm", bufs=2, space="PSUM"))
        po = ctx.enter_context(tc.tile_pool(name="po", bufs=3, space="PSUM"))

        A_sb = wpool.tile([128, NDC * ER], BF16, tag="A")
        nc.sync.dma_start(out=A_sb[:], in_=Aw[:])
        Wc_sb = wpool.tile([128, NDC * 9], BF16, tag="Wc")
        nc.sync.dma_start(out=Wc_sb[:], in_=Wc[:])
        REP_sb = wpool.tile([E, ER], F32, tag="REP")
        nc.sync.dma_start(out=REP_sb[:], in_=REPs[:])
        ONES_sb = wpool.tile([E, 1], F32, tag="ONES")
        nc.sync.dma_start(out=ONES_sb[:], in_=ONES8[:])
        SEL9_sb = wpool.tile([9, E], F32, tag="SEL9")
        nc.sync.dma_start(out=SEL9_sb[:], in_=SEL9[:])
        BC1_sb = wpool.tile([1, E], F32, tag="BC1")
        nc.sync.dma_start(out=BC1_sb[:], in_=BC1[:])
        BR_sb = wpool.tile([E, 1], F32, tag="BR")
        nc.sync.dma_start(out=BR_sb[:], in_=BR[:])
        BT8_sb = wpool.tile([E, 1], F32, tag="BT8")
        nc.sync.dma_start(out=BT8_sb[:], in_=BT8[:])
        # B is not needed until the first mm2 (~25us in); load it after the
        # small consts so they don't queue behind its 1MB on the sync ring
        B_sb = wpool.tile([ER, DOUT], BF16, tag="B")
        nc.sync.dma_start(out=B_sb[:], in_=Bl[:])

        def emit_block(t0, bs):
            x0 = NDC * t0
            ncols = NDC * bs
            xb = xpool.tile([128, ncols], BF16, tag="xb")
            # f32 -> bf16 cast inline in SWDGE DMA; 1 MB (f32 side) sub-DMAs
            for c0 in range(0, ncols, 2048):
                c1 = min(c0 + 2048, ncols)
                nc.gpsimd.dma_start(out=xb[:, c0:c1], in_=X[:, x0 + c0 : x0 + c1])

            # routing logits first so the routing chain (PE/DVE/ACT ping-pong)
            # overlaps the h accumulation
            r_ps = pr.tile([9, bs], F32, tag="r")
            for dc in range(NDC):
                nc.tensor.matmul(
                    r_ps[:],
                    Wc_sb[:, dc * 9 : (dc + 1) * 9],
                    xb[:, dc * bs : (dc + 1) * bs],
                    start=(dc == 0),
                    stop=(dc == NDC - 1),
                )
            # h[er, t] accumulated over the 32 d-chunks
            h_ps = ph.tile([ER, bs], F32, tag="h")
            for dc in range(NDC):
                nc.tensor.matmul(
                    h_ps[:],
                    A_sb[:, dc * ER : (dc + 1) * ER],
                    xb[:, dc * bs : (dc + 1) * bs],
                    start=(dc == 0),
                    stop=(dc == NDC - 1),
                )

            # ---- routing math (all partition-slices start at base 0) ----
            r_sb = spool.tile([9, bs], F32, tag="rsb")
            nc.scalar.activation(r_sb[:], r_ps[:], AF.Copy)
            eexp = spool.tile([E, bs], F32, tag="eexp")
            nc.scalar.activation(eexp[:], r_ps[0:E, :], AF.Exp, bias=BR_sb[:])
            S1 = pm.tile([1, bs], F32, tag="pm")
            nc.tensor.matmul(S1[:], ONES_sb[:], eexp[:], start=True, stop=True)
            sg = spool.tile([1, bs], F32, tag="sg")
            nc.vector.reciprocal(sg[:], S1[:])
            GD8 = pm.tile([E, bs], F32, tag="pm")
            nc.tensor.matmul(GD8[:], BC1_sb[:], sg[:], start=True, stop=True)
            t1 = spool.tile([E, bs], F32, tag="t1")
            nc.vector.tensor_mul(t1[:], eexp[:], GD8[:])  # gate
            RT8 = pm.tile([E, bs], F32, tag="pm")
            nc.tensor.matmul(RT8[:], SEL9_sb[:], r_sb[:], start=True, stop=True)
            th8 = spool.tile([E, bs], F32, tag="th8")
            nc.scalar.activation(th8[:], RT8[:], AF.Sigmoid, bias=BT8_sb[:])
            # adapted = gate - sigmoid(rt)/E  =  (th8 * -1/E) + t1
            adapted = spool.tile([E, bs], F32, tag="ad")
            nc.vector.scalar_tensor_tensor(
                adapted[:], th8[:], -1.0 / E, t1[:],
                mybir.AluOpType.mult, mybir.AluOpType.add,
            )
            wrelu = spool.tile([E, bs], F32, tag="wr")
            nc.vector.tensor_relu(wrelu[:], adapted[:])
            S2 = pm.tile([1, bs], F32, tag="pm")
            nc.tensor.matmul(S2[:], ONES_sb[:], wrelu[:], start=True, stop=True)
            smax = spool.tile([1, bs], F32, tag="sm")
            nc.vector.tensor_scalar_max(smax[:], S2[:], 1e-30)
            srecip = spool.tile([1, bs], F32, tag="sr")
            nc.vector.reciprocal(srecip[:], smax[:])
            SR8 = pm.tile([E, bs], F32, tag="pm")
            nc.tensor.matmul(SR8[:], BC1_sb[:], srecip[:], start=True, stop=True)
            wn = spool.tile([E, bs], F32, tag="wn")
            nc.vector.tensor_mul(wn[:], wrelu[:], SR8[:])
            WREPp = pm.tile([ER, bs], F32, tag="pm")
            nc.tensor.matmul(WREPp[:], REP_sb[:], wn[:], start=True, stop=True)
            WREP = spool.tile([ER, bs], F32, tag="WREP")
            nc.scalar.activation(WREP[:], WREPp[:], AF.Copy)
            hw = hwpool.tile([ER, bs], BF16, tag="hw")
            nc.vector.tensor_mul(hw[:], WREP[:], h_ps[:])

            # ---- second matmul + output ----
            # (HW A/B x3: spreading stores across the two HWDGE rings makes
            # no measurable difference on this device; keep the sync ring)
            for t4 in range(bs // 128):
                row = t0 + t4 * 128
                o_sb = opool.tile([128, DOUT], F32, tag="osb")
                for nb in range(NOB):
                    o_ps = po.tile([128, 512], F32, tag="o")
                    nc.tensor.matmul(
                        o_ps[:],
                        hw[:, t4 * 128 : (t4 + 1) * 128],
                        B_sb[:, nb * 512 : (nb + 1) * 512],
                        start=True,
                        stop=True,
                    )
                    if nb % 2 == 0:
                        nc.scalar.activation(
                            o_sb[:, nb * 512 : (nb + 1) * 512], o_ps[:], AF.Copy
                        )
                    else:
                        nc.vector.tensor_copy(
                            o_sb[:, nb * 512 : (nb + 1) * 512], o_ps[:]
                        )
                nc.sync.dma_start(out=OUT[row : row + 128, :], in_=o_sb[:])

        def emit_all():
            t0 = 0
            for bs in BLOCKS:
                emit_block(t0, bs)
                t0 += bs

        if loop:
            with tc.For_i(0, reps, 1):
                emit_all()
        else:
            for r in range(reps):
                emit_all()

    nc.compile()
    return nc


def _prep_consts(Wr, br, Wt, bt, A, Bw):
    bf = ml_dtypes.bfloat16
    A_all = np.ascontiguousarray(
        np.asarray(A, np.float32).transpose(1, 0, 2).reshape(D, ER)
    )  # [d, er]
    A_host = np.ascontiguousarray(
        A_all.reshape(NDC, 128, ER).transpose(1, 0, 2).reshape(128, NDC * ER)
    ).astype(bf)
    Wcat = np.concatenate(
        [np.asarray(Wr, np.float32).T, np.asarray(Wt, np.float32).T], axis=1
    )  # [d, 9]
    Wc_host = np.ascontiguousarray(
        Wcat.reshape(NDC, 128, 9).transpose(1, 0, 2).reshape(128, NDC * 9)
    ).astype(bf)
    B_host = np.ascontiguousarray(np.asarray(Bw, np.float32).reshape(ER, DOUT)).astype(bf)
    REPh = np.zeros((E, ER), np.float32)
    for e in range(E):
        REPh[e, e * R : (e + 1) * R] = SCALING
    ONESh = np.ones((E, 1), np.float32)
    SEL9h = np.zeros((9, E), np.float32)
    SEL9h[8, :] = 1.0
    BC1h = np.ones((1, E), np.float32)
    BRh = np.asarray(br, np.float32).reshape(E, 1)
    BT8h = np.full((E, 1), np.float32(np.asarray(bt).reshape(())), np.float32)
    return {
        "Aw": A_host,
        "Wc": Wc_host,
        "Bl": B_host,
        "REPs": REPh,
        "ONES8": ONESh,
        "SEL9": SEL9h,
        "BC1": BC1h,
        "BR": BRh,
        "BT8": BT8h,
    }


def _prep_x(xs):
    """Per-core shard [NTOK, D] -> [128, sum(NDC*bs)] with per-block
    [p, dc, t] layout so every DMA slice is contiguous."""
    parts = []
    t0 = 0
    for bs in BLOCKS:
        blkarr = (
            xs[t0 : t0 + bs].reshape(bs, NDC, 128).transpose(2, 1, 0).reshape(128, NDC * bs)
        )
        parts.append(blkarr)
        t0 += bs
    return np.ascontiguousarray(np.concatenate(parts, axis=1))


def kernel(x, Wr, br, Wt, bt, A, Bw, _trace=False, _trace_kwargs=None):
    if "nc" not in _CACHE:
        _CACHE["nc"] = _build()
    nc = _CACHE["nc"]

    consts = _prep_consts(Wr, br, Wt, bt, A, Bw)
    xf = np.asarray(x, np.float32).reshape(N, D)
    in_maps = []
    for c in range(NCORES):
        Xh = _prep_x(xf[c * NTOK : (c + 1) * NTOK])
        in_maps.append({"X": Xh, **consts})

    res = run_bass_kernel_spmd(
        nc,
        in_maps,
        core_ids=list(range(NCORES)),
        trace=_trace,
        **(_trace_kwargs or {}),
    )
    out = np.concatenate([res.results[c]["out"] for c in range(NCORES)], axis=0)
    if _trace:
        _CACHE["last_res"] = res
    return out.reshape(B, S, DOUT).astype(np.float32)



# revision 2
# speedup vs baseline: 1.0318x; 1.0318x over previous
"""AdaMoLE (LoRA-MoE routing) Trainium2 kernel, data-parallel over tokens on 8 cores.

Math (per token n):
    logits = x @ Wr.T + br                 [E]
    gate   = softmax(logits)
    thr    = sigmoid(x @ Wt.T + bt) / E    [1]
    w      = relu(gate - thr); w /= max(sum(w), eps)
    h      = x @ A_all                     [E*R]   (A_all = concat_e A_e)
    out    = (h * rep(w) * SCALING) @ B_all        (B_all = concat_e B_e)

Each core takes 2048 tokens, processed in 512-token blocks. x is passed
host-side in a transposed, chunk-major bf16 layout so the contraction dim
lands on SBUF partitions and every DMA is a large contiguous transfer
(bf16 halves the HBM read vs f32+inline-cast). The output is written bf16
in a block-major device layout and unshuffled + upcast to f32 on the host.
Rings: X loads on sync (HWDGE), output stores on scalar (HWDGE), weights
on gpsimd (SWDGE) so the three streams don't serialize behind each other.
Cross-expert (partition-dim) softmax reductions use tiny PE ones-matmuls;
routing weight broadcast across the er=128 lanes via a replication matmul.
"""

import sys

sys.path.insert(0, "/opt/trn_rl_repo")

import numpy as np
import ml_dtypes

import concourse.bacc as bacc
import concourse.mybir as mybir
import concourse.tile as tile
from concourse.bass_utils import run_bass_kernel_spmd
from contextlib import ExitStack

F32 = mybir.dt.float32
BF16 = mybir.dt.bfloat16
AF = mybir.ActivationFunctionType

B, S, D, DOUT = 4, 4096, 4096, 4096
R, E = 16, 8
SCALING = 8.0 / R  # lora_alpha / r
NCORES = 8
N = B * S
NTOK = N // NCORES        # 2048 tokens per core
TBS = 512                 # tokens per block
NBLK = NTOK // TBS        # 4 blocks
NDC = D // 128            # 32 contraction chunks
ER = E * R                # 128
CPB = NDC * TBS           # X columns per block (bf16)
NG = TBS // 128           # 128-token groups per block
OPB = NG * DOUT           # OUT columns per block (bf16)
LSUB = 2                  # sub-DMAs per X block load
SSUB = 2                  # sub-DMAs per OUT block store

_CACHE = {}


def _build(reps=1, loop=False):
    nc = bacc.Bacc("TRN2", debug=False, num_devices=NCORES)

    X = nc.declare_dram_parameter("X", [128, NBLK * CPB], BF16, isOutput=False)
    Aw = nc.declare_dram_parameter("Aw", [128, NDC * ER], BF16, isOutput=False)
    Wc = nc.declare_dram_parameter("Wc", [128, NDC * 9], BF16, isOutput=False)
    Bl = nc.declare_dram_parameter("Bl", [ER, DOUT], BF16, isOutput=False)
    REPs = nc.declare_dram_parameter("REPs", [E, ER], F32, isOutput=False)
    ONES8 = nc.declare_dram_parameter("ONES8", [E, 1], F32, isOutput=False)
    SEL9 = nc.declare_dram_parameter("SEL9", [9, E], F32, isOutput=False)
    BC1 = nc.declare_dram_parameter("BC1", [1, E], F32, isOutput=False)
    BR = nc.declare_dram_parameter("BR", [E, 1], F32, isOutput=False)
    BT8 = nc.declare_dram_parameter("BT8", [E, 1], F32, isOutput=False)
    OUT = nc.declare_dram_parameter("out", [128, NBLK * OPB], BF16, isOutput=True)

    with tile.TileContext(nc) as tc, ExitStack() as ctx:
        wpool = ctx.enter_context(tc.tile_pool(name="w", bufs=1))
        xpool = ctx.enter_context(tc.tile_pool(name="x", bufs=2))
        opool = ctx.enter_context(tc.tile_pool(name="o", bufs=2))
        spool = ctx.enter_context(tc.tile_pool(name="s", bufs=2))
        hwpool = ctx.enter_context(tc.tile_pool(name="hw", bufs=2))
        ph = ctx.enter_context(tc.tile_pool(name="ph", bufs=2, space="PSUM"))
        pr = ctx.enter_context(tc.tile_pool(name="pr", bufs=1, space="PSUM"))
        pm = ctx.enter_context(tc.tile_pool(name="pm", bufs=2, space="PSUM"))
        po = ctx.enter_context(tc.tile_pool(name="po", bufs=3, space="PSUM"))

        # weights on the SWDGE (gpsimd) ring: X loads start immediately on the
        # sync ring without queueing behind 2 MB of weights
        Wc_sb = wpool.tile([128, NDC * 9], BF16, tag="Wc")
        nc.gpsimd.dma_start(out=Wc_sb[:], in_=Wc[:])
        A_sb = wpool.tile([128, NDC * ER], BF16, tag="A")
        nc.gpsimd.dma_start(out=A_sb[:], in_=Aw[:])
        REP_sb = wpool.tile([E, ER], F32, tag="REP")
        nc.gpsimd.dma_start(out=REP_sb[:], in_=REPs[:])
        ONES_sb = wpool.tile([E, 1], F32, tag="ONES")
        nc.gpsimd.dma_start(out=ONES_sb[:], in_=ONES8[:])
        SEL9_sb = wpool.tile([9, E], F32, tag="SEL9")
        nc.gpsimd.dma_start(out=SEL9_sb[:], in_=SEL9[:])
        BC1_sb = wpool.tile([1, E], F32, tag="BC1")
        nc.gpsimd.dma_start(out=BC1_sb[:], in_=BC1[:])
        BR_sb = wpool.tile([E, 1], F32, tag="BR")
        nc.gpsimd.dma_start(out=BR_sb[:], in_=BR[:])
        BT8_sb = wpool.tile([E, 1], F32, tag="BT8")
        nc.gpsimd.dma_start(out=BT8_sb[:], in_=BT8[:])
        B_sb = wpool.tile([ER, DOUT], BF16, tag="B")
        nc.gpsimd.dma_start(out=B_sb[:], in_=Bl[:])

        def emit_block(blk):
            x0 = blk * CPB
            xb = xpool.tile([128, CPB], BF16, tag="xb")
            for c0 in range(0, CPB, CPB // LSUB):
                c1 = c0 + CPB // LSUB
                nc.sync.dma_start(out=xb[:, c0:c1], in_=X[:, x0 + c0 : x0 + c1])

            # routing logits first so the routing chain (PE/DVE/ACT ping-pong)
            # overlaps the h accumulation
            r_ps = pr.tile([9, TBS], F32, tag="r")
            for dc in range(NDC):
                nc.tensor.matmul(
                    r_ps[:],
                    Wc_sb[:, dc * 9 : (dc + 1) * 9],
                    xb[:, dc * TBS : (dc + 1) * TBS],
                    start=(dc == 0),
                    stop=(dc == NDC - 1),
                )
            # h[er, t] accumulated over the 32 d-chunks
            h_ps = ph.tile([ER, TBS], F32, tag="h")
            for dc in range(NDC):
                nc.tensor.matmul(
                    h_ps[:],
                    A_sb[:, dc * ER : (dc + 1) * ER],
                    xb[:, dc * TBS : (dc + 1) * TBS],
                    start=(dc == 0),
                    stop=(dc == NDC - 1),
                )

            # ---- routing math (all partition-slices start at base 0) ----
            r_sb = spool.tile([9, TBS], F32, tag="rsb")
            nc.scalar.activation(r_sb[:], r_ps[:], AF.Copy)
            eexp = spool.tile([E, TBS], F32, tag="eexp")
            nc.scalar.activation(eexp[:], r_ps[0:E, :], AF.Exp, bias=BR_sb[:])
            S1 = pm.tile([1, TBS], F32, tag="pm")
            nc.tensor.matmul(S1[:], ONES_sb[:], eexp[:], start=True, stop=True)
            sg = spool.tile([1, TBS], F32, tag="sg")
            nc.vector.reciprocal(sg[:], S1[:])
            GD8 = pm.tile([E, TBS], F32, tag="pm")
            nc.tensor.matmul(GD8[:], BC1_sb[:], sg[:], start=True, stop=True)
            t1 = spool.tile([E, TBS], F32, tag="t1")
            nc.vector.tensor_mul(t1[:], eexp[:], GD8[:])  # gate
            RT8 = pm.tile([E, TBS], F32, tag="pm")
            nc.tensor.matmul(RT8[:], SEL9_sb[:], r_sb[:], start=True, stop=True)
            th8 = spool.tile([E, TBS], F32, tag="th8")
            nc.scalar.activation(th8[:], RT8[:], AF.Sigmoid, bias=BT8_sb[:])
            # adapted = gate - sigmoid(rt)/E  =  (th8 * -1/E) + t1
            adapted = spool.tile([E, TBS], F32, tag="ad")
            nc.vector.scalar_tensor_tensor(
                adapted[:], th8[:], -1.0 / E, t1[:],
                mybir.AluOpType.mult, mybir.AluOpType.add,
            )
            wrelu = spool.tile([E, TBS], F32, tag="wr")
            nc.vector.tensor_relu(wrelu[:], adapted[:])
            S2 = pm.tile([1, TBS], F32, tag="pm")
            nc.tensor.matmul(S2[:], ONES_sb[:], wrelu[:], start=True, stop=True)
            smax = spool.tile([1, TBS], F32, tag="sm")
            nc.vector.tensor_scalar_max(smax[:], S2[:], 1e-30)
            srecip = spool.tile([1, TBS], F32, tag="sr")
            nc.vector.reciprocal(srecip[:], smax[:])
            SR8 = pm.tile([E, TBS], F32, tag="pm")
            nc.tensor.matmul(SR8[:], BC1_sb[:], srecip[:], start=True, stop=True)
            wn = spool.tile([E, TBS], F32, tag="wn")
            nc.vector.tensor_mul(wn[:], wrelu[:], SR8[:])
            WREPp = pm.tile([ER, TBS], F32, tag="pm")
            nc.tensor.matmul(WREPp[:], REP_sb[:], wn[:], start=True, stop=True)
            WREP = spool.tile([ER, TBS], F32, tag="WREP")
            nc.scalar.activation(WREP[:], WREPp[:], AF.Copy)
            hw = hwpool.tile([ER, TBS], BF16, tag="hw")
            nc.vector.tensor_mul(hw[:], WREP[:], h_ps[:])

            # ---- second matmul + output (bf16, block-major layout) ----
            o_sb = opool.tile([128, OPB], BF16, tag="osb")
            for t4 in range(NG):
                for nb in range(DOUT // 512):
                    o_ps = po.tile([128, 512], F32, tag="o")
                    nc.tensor.matmul(
                        o_ps[:],
                        hw[:, t4 * 128 : (t4 + 1) * 128],
                        B_sb[:, nb * 512 : (nb + 1) * 512],
                        start=True,
                        stop=True,
                    )
                    dst = o_sb[:, t4 * DOUT + nb * 512 : t4 * DOUT + (nb + 1) * 512]
                    if nb % 2 == 0:
                        nc.scalar.activation(dst, o_ps[:], AF.Copy)
                    else:
                        nc.vector.tensor_copy(dst, o_ps[:])
            for c0 in range(0, OPB, OPB // SSUB):
                c1 = c0 + OPB // SSUB
                nc.scalar.dma_start(
                    out=OUT[:, blk * OPB + c0 : blk * OPB + c1], in_=o_sb[:, c0:c1]
                )

        def emit_all():
            for blk in range(NBLK):
                emit_block(blk)

        if loop:
            with tc.For_i(0, reps, 1):
                emit_all()
        else:
            for r in range(reps):
                emit_all()

    nc.compile()
    return nc


def _prep_consts(Wr, br, Wt, bt, A, Bw):
    bf = ml_dtypes.bfloat16
    A_all = np.ascontiguousarray(
        np.asarray(A, np.float32).transpose(1, 0, 2).reshape(D, ER)
    )  # [d, er]
    A_host = np.ascontiguousarray(
        A_all.reshape(NDC, 128, ER).transpose(1, 0, 2).reshape(128, NDC * ER)
    ).astype(bf)
    Wcat = np.concatenate(
        [np.asarray(Wr, np.float32).T, np.asarray(Wt, np.float32).T], axis=1
    )  # [d, 9]
    Wc_host = np.ascontiguousarray(
        Wcat.reshape(NDC, 128, 9).transpose(1, 0, 2).reshape(128, NDC * 9)
    ).astype(bf)
    B_host = np.ascontiguousarray(np.asarray(Bw, np.float32).reshape(ER, DOUT)).astype(bf)
    REPh = np.zeros((E, ER), np.float32)
    for e in range(E):
        REPh[e, e * R : (e + 1) * R] = SCALING
    ONESh = np.ones((E, 1), np.float32)
    SEL9h = np.zeros((9, E), np.float32)
    SEL9h[8, :] = 1.0
    BC1h = np.ones((1, E), np.float32)
    BRh = np.asarray(br, np.float32).reshape(E, 1)
    BT8h = np.full((E, 1), np.float32(np.asarray(bt).reshape(())), np.float32)
    return {
        "Aw": A_host,
        "Wc": Wc_host,
        "Bl": B_host,
        "REPs": REPh,
        "ONES8": ONESh,
        "SEL9": SEL9h,
        "BC1": BC1h,
        "BR": BRh,
        "BT8": BT8h,
    }


def _prep_x(xs):
    """Per-core shard [NTOK, D] (bf16) -> [128, NBLK*CPB] with per-block
    [p, dc, t] layout so every DMA slice is contiguous."""
    parts = []
    for blk in range(NBLK):
        t0 = blk * TBS
        blkarr = (
            xs[t0 : t0 + TBS]
            .reshape(TBS, NDC, 128)
            .transpose(2, 1, 0)
            .reshape(128, CPB)
        )
        parts.append(blkarr)
    return np.ascontiguousarray(np.concatenate(parts, axis=1))


def _unshard_out(oarr):
    """Device layout [128, NBLK*OPB] (bf16) -> [NTOK, DOUT] f32."""
    o = np.asarray(oarr).reshape(128, NBLK, NG, DOUT)
    return o.transpose(1, 2, 0, 3).reshape(NTOK, DOUT).astype(np.float32)


def kernel(x, Wr, br, Wt, bt, A, Bw, _trace=False, _trace_kwargs=None):
    if "nc" not in _CACHE:
        _CACHE["nc"] = _build()
    nc = _CACHE["nc"]

    consts = _prep_consts(Wr, br, Wt, bt, A, Bw)
    xf = np.asarray(x, np.float32).reshape(N, D).astype(ml_dtypes.bfloat16)
    in_maps = []
    for c in range(NCORES):
        Xh = _prep_x(xf[c * NTOK : (c + 1) * NTOK])
        in_maps.append({"X": Xh, **consts})

    res = run_bass_kernel_spmd(
        nc,
        in_maps,
        core_ids=list(range(NCORES)),
        trace=_trace,
        **(_trace_kwargs or {}),
    )
    out = np.concatenate(
        [_unshard_out(res.results[c]["out"]) for c in range(NCORES)], axis=0
    )
    if _trace:
        _CACHE["last_res"] = res
    return out.reshape(B, S, DOUT)


# revision 9
# speedup vs baseline: 3.3466x; 3.2436x over previous
"""AdaMoLE (LoRA-MoE routing) Trainium2 kernel, data-parallel over tokens on 8 cores.

Math (per token n):
    logits = x @ Wr.T + br                 [E]
    gate   = softmax(logits)
    thr    = sigmoid(x @ Wt.T + bt) / E    [1]
    w      = relu(gate - thr); w /= max(sum(w), eps)
    h      = x @ A_all                     [E*R]   (A_all = concat_e A_e)
    out    = (h * rep(w) * SCALING) @ B_all        (B_all = concat_e B_e)

Key restructurings vs the straightforward version:
  * Scale cancellation: w = relu(gate - thr)/sum(...) is invariant to the
    softmax denominator S, so we use w' = relu(eexp - thr*S) and divide by
    sum(w') instead -- no gate normalization broadcast needed.
  * The final 1/sum(w') is applied AFTER the second matmul, where tokens sit
    on partitions, as a per-partition scalar in the PSUM->SBUF copy.
  * All PE operands are bf16 (f32 matmuls run 4x slower); sigmoid goes
    through Exp so the scalar engine keeps one activation table loaded.
  * Software-pipelined emission: block k+1's router matmuls are emitted
    inside block k's routing chain so the PE never waits on DVE latency.

Each core takes 2048 tokens in 4 x 512-token blocks. x arrives host-side
pre-cast to bf16 in a transposed chunk-major layout (halves HBM read);
the output is written bf16 in a block-major layout and unshuffled + upcast
on the host. Rings: X loads on sync (HWDGE), stores on scalar (HWDGE),
weights on gpsimd (SWDGE).
"""

import sys

sys.path.insert(0, "/opt/trn_rl_repo")

import numpy as np
import ml_dtypes

import concourse.bacc as bacc
import concourse.mybir as mybir
import concourse.tile as tile
from concourse.bass_utils import run_bass_kernel_spmd
from contextlib import ExitStack

F32 = mybir.dt.float32
BF16 = mybir.dt.bfloat16
AF = mybir.ActivationFunctionType
ALU = mybir.AluOpType

B, S, D, DOUT = 4, 4096, 4096, 4096
R, E = 16, 8
SCALING = 8.0 / R  # lora_alpha / r
NCORES = 8
N = B * S
NTOK = N // NCORES        # 2048 tokens per core
TBS = 512                 # tokens per block
NBLK = NTOK // TBS        # 4 blocks
NDC = D // 128            # 32 contraction chunks
ER = E * R                # 128
CPB = NDC * TBS           # X columns per block (bf16)
NG = TBS // 128           # 128-token groups per block
OPB = NG * DOUT           # OUT columns per block (bf16)
SSUB = 2                  # sub-DMAs per OUT block store

_CACHE = {}


def _build(reps=1, loop=False):
    nc = bacc.Bacc("TRN2", debug=False, num_devices=NCORES)

    X = nc.declare_dram_parameter("X", [128, NBLK * CPB], BF16, isOutput=False)
    Aw = nc.declare_dram_parameter("Aw", [128, NDC * ER], BF16, isOutput=False)
    Wc = nc.declare_dram_parameter("Wc", [128, NDC * 9], BF16, isOutput=False)
    Bl = nc.declare_dram_parameter("Bl", [ER, DOUT], BF16, isOutput=False)
    # SMB packs the small bf16 consts: [0:8,0:128]=REP (w->er replicate, pre-
    # scaled by SCALING), [0:8,128:129]=ones[8,1], [0,129:137]=ones[1,8],
    # [0:9,137:138]=row-8 selector
    SMB = nc.declare_dram_parameter("SMB", [9, 138], BF16, isOutput=False)
    # CB9 packs the Exp prologue: [:,0]=bias (br; -bt), [:,1]=scale (1x8; -1)
    CB9 = nc.declare_dram_parameter("CB9", [9, 2], F32, isOutput=False)
    OUT = nc.declare_dram_parameter("out", [128, NBLK * OPB], BF16, isOutput=True)

    with tile.TileContext(nc) as tc, ExitStack() as ctx:
        wpool = ctx.enter_context(tc.tile_pool(name="w", bufs=1))
        xpool = ctx.enter_context(tc.tile_pool(name="x", bufs=2))
        opool = ctx.enter_context(tc.tile_pool(name="o", bufs=2))
        spool = ctx.enter_context(tc.tile_pool(name="s", bufs=2))
        hwpool = ctx.enter_context(tc.tile_pool(name="hw", bufs=2))
        ph = ctx.enter_context(tc.tile_pool(name="ph", bufs=2, space="PSUM"))
        pr = ctx.enter_context(tc.tile_pool(name="pr", bufs=1, space="PSUM"))
        pm = ctx.enter_context(tc.tile_pool(name="pm", bufs=2, space="PSUM"))
        po = ctx.enter_context(tc.tile_pool(name="po", bufs=3, space="PSUM"))

        # weights on the SWDGE (gpsimd) ring: X loads start immediately on the
        # sync ring without queueing behind 2 MB of weights
        Wc_sb = wpool.tile([128, NDC * 9], BF16, tag="Wc")
        nc.gpsimd.dma_start(out=Wc_sb[:], in_=Wc[:])
        A_sb = wpool.tile([128, NDC * ER], BF16, tag="A")
        nc.gpsimd.dma_start(out=A_sb[:], in_=Aw[:])
        SMB_sb = wpool.tile([9, 138], BF16, tag="SMB")
        nc.gpsimd.dma_start(out=SMB_sb[:], in_=SMB[:])
        CB9_sb = wpool.tile([9, 2], F32, tag="CB9")
        nc.gpsimd.dma_start(out=CB9_sb[:], in_=CB9[:])
        B_sb = wpool.tile([ER, DOUT], BF16, tag="B")
        nc.gpsimd.dma_start(out=B_sb[:], in_=Bl[:])

        REPb = SMB_sb[0:8, 0:128]
        ONESb = SMB_sb[0:8, 128:129]
        BC1b = SMB_sb[0:1, 129:137]
        SEL9b = SMB_sb[0:9, 137:138]
        BIAS9 = CB9_sb[:, 0:1]
        SCL9 = CB9_sb[:, 1:2]

        xb_t = [None] * NBLK
        rps_t = [None] * NBLK

        def emit_load(k):
            x0 = k * CPB
            xb = xpool.tile([128, CPB], BF16, tag="xb")
            xb_t[k] = xb
            # block 0 split finer so the PE starts sooner (pipeline fill)
            subs = [CPB // 4, CPB // 4, CPB // 2] if k == 0 else [CPB]
            c0 = 0
            for w in subs:
                nc.sync.dma_start(out=xb[:, c0 : c0 + w], in_=X[:, x0 + c0 : x0 + c0 + w])
                c0 += w

        def emit_r(k):
            r_ps = pr.tile([9, TBS], F32, tag="r")
            rps_t[k] = r_ps
            xb = xb_t[k]
            for dc in range(NDC):
                nc.tensor.matmul(
                    r_ps[:],
                    Wc_sb[:, dc * 9 : (dc + 1) * 9],
                    xb[:, dc * TBS : (dc + 1) * TBS],
                    start=(dc == 0),
                    stop=(dc == NDC - 1),
                )

        def emit_block(k):
            xb = xb_t[k]
            r_ps = rps_t[k]

            # h[er, t] accumulated over the 32 d-chunks
            h_ps = ph.tile([ER, TBS], F32, tag="h")
            for dc in range(NDC):
                nc.tensor.matmul(
                    h_ps[:],
                    A_sb[:, dc * ER : (dc + 1) * ER],
                    xb[:, dc * TBS : (dc + 1) * TBS],
                    start=(dc == 0),
                    stop=(dc == NDC - 1),
                )

            # ---- routing, front half ----
            # e9[0:8] = exp(logits + br)  (softmax numerator)
            # e9[8]   = exp(-(rt + bt))   (for sigmoid via exp)
            e9 = spool.tile([9, TBS], BF16, tag="e9")
            nc.scalar.activation(e9[:], r_ps[:], AF.Exp, bias=BIAS9, scale=SCL9)
            S1 = pm.tile([1, TBS], F32, tag="pm")
            nc.tensor.matmul(S1[:], ONESb, e9[0:8, :], start=True, stop=True)
            # DVE can't read at partition base 8: extract e9 row 8 (=exp(-rt-bt))
            # to partition 0 with a selector matmul
            en0 = pm.tile([1, TBS], F32, tag="pm")
            nc.tensor.matmul(en0[:], SEL9b, e9[:], start=True, stop=True)
            # den = E * (1 + exp(-(rt+bt)))  ==  E / sigmoid(rt+bt)
            den = spool.tile([1, TBS], F32, tag="den")
            nc.vector.tensor_scalar(den[:], en0[:], 1.0, float(E), ALU.add, ALU.mult)
            rec = spool.tile([1, TBS], F32, tag="rec")
            nc.vector.reciprocal(rec[:], den[:])
            # thrS = sigmoid(rt+bt)/E * S  (threshold in the unnormalized space)
            thrS = spool.tile([1, TBS], BF16, tag="thrS")
            nc.vector.tensor_mul(thrS[:], S1[:], rec[:])
            TH8 = pm.tile([8, TBS], F32, tag="pm")
            nc.tensor.matmul(TH8[:], BC1b, thrS[:], start=True, stop=True)

            # block k+1's router matmuls slot in here: the PE chews on them
            # while the DVE finishes this block's routing chain
            if k + 2 < NBLK:
                emit_load(k + 2)
            if k + 1 < NBLK:
                emit_r(k + 1)

            # ---- routing, back half ----
            wsub = spool.tile([8, TBS], F32, tag="wsub")
            nc.vector.scalar_tensor_tensor(
                wsub[:], TH8[:], -1.0, e9[0:8, :], ALU.mult, ALU.add
            )
            wrelu = spool.tile([8, TBS], BF16, tag="wrelu")
            nc.vector.tensor_scalar_max(wrelu[:], wsub[:], 0.0)
            # per-token sum of selected weights, tokens on partitions
            S2T = pm.tile([128, NG], F32, tag="pm")
            for g in range(NG):
                nc.tensor.matmul(
                    S2T[:, g : g + 1],
                    wrelu[:, g * 128 : (g + 1) * 128],
                    ONESb,
                    start=True,
                    stop=True,
                )
            clmp = spool.tile([128, NG], F32, tag="clmp")
            nc.vector.tensor_scalar_max(clmp[:], S2T[:], 1e-30)
            srecT = spool.tile([128, NG], F32, tag="srecT")
            nc.vector.reciprocal(srecT[:], clmp[:])
            WREPp = pm.tile([ER, TBS], F32, tag="pm")
            nc.tensor.matmul(WREPp[:], REPb, wrelu[:], start=True, stop=True)
            WREP = spool.tile([ER, TBS], BF16, tag="WREP")
            nc.scalar.activation(WREP[:], WREPp[:], AF.Copy)
            hw = hwpool.tile([ER, TBS], BF16, tag="hw")
            nc.vector.tensor_mul(hw[:], WREP[:], h_ps[:])

            # ---- second matmul + scaled output copy (bf16, block-major) ----
            o_sb = opool.tile([128, OPB], BF16, tag="osb")
            for t4 in range(NG):
                sc = srecT[:, t4 : t4 + 1]
                for nb in range(DOUT // 512):
                    o_ps = po.tile([128, 512], F32, tag="o")
                    nc.tensor.matmul(
                        o_ps[:],
                        hw[:, t4 * 128 : (t4 + 1) * 128],
                        B_sb[:, nb * 512 : (nb + 1) * 512],
                        start=True,
                        stop=True,
                    )
                    dst = o_sb[:, t4 * DOUT + nb * 512 : t4 * DOUT + (nb + 1) * 512]
                    if nb % 2 == 0:
                        nc.scalar.activation(dst, o_ps[:], AF.Copy, scale=sc)
                    else:
                        nc.vector.tensor_scalar_mul(dst, o_ps[:], sc)
            for c0 in range(0, OPB, OPB // SSUB):
                c1 = c0 + OPB // SSUB
                nc.scalar.dma_start(
                    out=OUT[:, k * OPB + c0 : k * OPB + c1], in_=o_sb[:, c0:c1]
                )

        def emit_all():
            emit_load(0)
            if NBLK > 1:
                emit_load(1)
            emit_r(0)
            for k in range(NBLK):
                emit_block(k)

        if loop:
            with tc.For_i(0, reps, 1):
                emit_all()
        else:
            for r in range(reps):
                emit_all()

    nc.compile()
    return nc


def _prep_consts(Wr, br, Wt, bt, A, Bw):
    bf = ml_dtypes.bfloat16
    A_all = np.ascontiguousarray(
        np.asarray(A, np.float32).transpose(1, 0, 2).reshape(D, ER)
    )  # [d, er]
    A_host = np.ascontiguousarray(
        A_all.reshape(NDC, 128, ER).transpose(1, 0, 2).reshape(128, NDC * ER)
    ).astype(bf)
    Wcat = np.concatenate(
        [np.asarray(Wr, np.float32).T, np.asarray(Wt, np.float32).T], axis=1
    )  # [d, 9]
    Wc_host = np.ascontiguousarray(
        Wcat.reshape(NDC, 128, 9).transpose(1, 0, 2).reshape(128, NDC * 9)
    ).astype(bf)
    B_host = np.ascontiguousarray(np.asarray(Bw, np.float32).reshape(ER, DOUT)).astype(bf)
    SMBh = np.zeros((9, 138), np.float32)
    for e in range(E):
        SMBh[e, e * R : (e + 1) * R] = SCALING
    SMBh[0:8, 128] = 1.0
    SMBh[0, 129:137] = 1.0
    SMBh[8, 137] = 1.0
    CB9h = np.zeros((9, 2), np.float32)
    CB9h[0:8, 0] = np.asarray(br, np.float32).reshape(E)
    CB9h[8, 0] = -np.float32(np.asarray(bt).reshape(()))
    CB9h[0:8, 1] = 1.0
    CB9h[8, 1] = -1.0
    return {
        "Aw": A_host,
        "Wc": Wc_host,
        "Bl": B_host,
        "SMB": SMBh.astype(bf),
        "CB9": CB9h,
    }


def _prep_x(xs):
    """Per-core shard [NTOK, D] (bf16) -> [128, NBLK*CPB] with per-block
    [p, dc, t] layout so every DMA slice is contiguous."""
    parts = []
    for blk in range(NBLK):
        t0 = blk * TBS
        blkarr = (
            xs[t0 : t0 + TBS]
            .reshape(TBS, NDC, 128)
            .transpose(2, 1, 0)
            .reshape(128, CPB)
        )
        parts.append(blkarr)
    return np.ascontiguousarray(np.concatenate(parts, axis=1))


def _unshard_out(oarr):
    """Device layout [128, NBLK*OPB] (bf16) -> [NTOK, DOUT] f32."""
    o = np.asarray(oarr).reshape(128, NBLK, NG, DOUT)
    return o.transpose(1, 2, 0, 3).reshape(NTOK, DOUT).astype(np.float32)


def kernel(x, Wr, br, Wt, bt, A, Bw, _trace=False, _trace_kwargs=None):
    if "nc" not in _CACHE:
        _CACHE["nc"] = _build()
    nc = _CACHE["nc"]

    consts = _prep_consts(Wr, br, Wt, bt, A, Bw)
    xf = np.asarray(x, np.float32).reshape(N, D).astype(ml_dtypes.bfloat16)
    in_maps = []
    for c in range(NCORES):
        Xh = _prep_x(xf[c * NTOK : (c + 1) * NTOK])
        in_maps.append({"X": Xh, **consts})

    res = run_bass_kernel_spmd(
        nc,
        in_maps,
        core_ids=list(range(NCORES)),
        trace=_trace,
        **(_trace_kwargs or {}),
    )
    out = np.concatenate(
        [_unshard_out(res.results[c]["out"]) for c in range(NCORES)], axis=0
    )
    if _trace:
        _CACHE["last_res"] = res
    return out.reshape(B, S, DOUT)
